# revision 6
# baseline (speedup 1.0000x reference)
"""MultiHeadCrossAttention Trainium2 kernel (8 NeuronCores, SPMD).

Sharding: core c -> (n = c // 2, g = c % 2): one query batch n, half the
heads (8 of 16, embed slice g*512:(g+1)*512). Host compacts kv along KLEN
by the per-n mask (~50% survive), pads to KC = 128*T.

Key insight driving dtype choices: attention output is a softmax-weighted
AVERAGE of v, whose magnitude shrinks by the same sqrt(Neff) as the
weight-noise averaging gain -- so final rel err ~= per-weight RMS error.
fp8 weights (3-5% RMS) can never pass rel<2e-2; everything on the weight
and value paths stays bf16 (~0.2-1.8% RMS).

Per-core design (v3):
  - Projections (bf16, K=512 via 4 accumulating matmuls) flow through the
    shared PSUM pipeline; outputs evacuated fp32->bf16 by whichever of
    ScalarE/VectorE is currently less loaded (greedy balance).
  - Energy e[k,q] = kT.T @ qT per head at K=64 with NO zero padding: the
    two heads of a pair run as row-tiled concurrent matmuls (rows 0-63 /
    64-127 of the PE array) -> 2x the padded-K=128 baseline throughput.
  - exp is split across BOTH ScalarE and VectorE (the bottleneck is
    PSUM->SBUF evacuation bandwidth, exp included):
      * ScalarE: native Exp ACTIVATE, fp32 PSUM -> bf16 SBUF.
      * VectorE: one-op Schraudolph fast-exp: i16 = round(e*S1 + S2)
        written as int16 bits that ARE the bf16 encoding of ~exp(e/8)
        (fp32->int convert is round-to-nearest on HW; ~1.8% RMS, mean
        calibrated to zero via S2). One tensor_scalar per unit.
  - AV: per k-tile bf16 matmuls, lhsT = vsb [Ki=128 tokens, M=65] =
    [64 v-dims | indicator], indicator row accumulates the softmax
    denominator; pad tokens have v=0, ind=0. M=65 costs nothing (matmul
    time is set by N).
  - Unnormalized AV + denominator row DMA'd out; host divides/assembles.
"""

import math
import sys
from contextlib import ExitStack

import numpy as np

for _p in ("/opt/trn_rl_repo",):
    if _p not in sys.path:
        sys.path.insert(0, _p)

import ml_dtypes

import concourse.bass as bass  # noqa: F401  (import registers lowering deps)
import concourse.tile as tile
from concourse import bacc, mybir
from concourse.bass_utils import run_bass_kernel_spmd

BF16 = ml_dtypes.bfloat16

N, QLEN, KLEN = 4, 2048, 2048
QDIM = KVDIM = 512
EMBED, HEADS = 1024, 16
HEAD_DIM = 64
N_CORES = 8
SCALE = 1.0 / math.sqrt(HEAD_DIM)  # 1/8
# VectorE Schraudolph constants for bf16 bits (128/octave, bias 127):
#   i16 = round(e * (128/ln2)/8 + (127*128 - c))), c calibrated so the
#   piecewise-linear overestimate is mean-zero.
S1_DVE = (128.0 / math.log(2.0)) / 8.0
S2_DVE = 127.0 * 128.0 - 7.37

_cache: dict = {}
last_exec_time_ns = None
last_results = None


class _Balance:
    """Greedy ScalarE/VectorE assignment by simulated busy time."""

    def __init__(self):
        self.t_act = 0.0
        self.t_dve = 0.0

    @staticmethod
    def cost_act(fd):
        return (172.0 + fd) / 1.2

    @staticmethod
    def cost_dve(fd):
        return (120.0 + fd) / 0.96

    def pick(self, fd):
        ca = self.cost_act(fd)
        cd = self.cost_dve(fd)
        if self.t_act + ca <= self.t_dve + cd:
            self.t_act += ca
            return "act"
        self.t_dve += cd
        return "dve"


def _build(T: int, ql: int = QLEN):
    KC = 128 * T
    NQB = ql // 512
    dt = mybir.dt
    nc = bacc.Bacc("TRN2", target_bir_lowering=False, debug=False)

    qT_d = nc.dram_tensor("qt", [QDIM, ql], dt.bfloat16, kind="ExternalInput").ap()
    kT_d = nc.dram_tensor("kt", [KVDIM, KC], dt.bfloat16, kind="ExternalInput").ap()
    vT_d = nc.dram_tensor("vt", [KVDIM, KC], dt.bfloat16, kind="ExternalInput").ap()
    wq_d = nc.dram_tensor("wq", [QDIM, 512], dt.bfloat16, kind="ExternalInput").ap()
    wk_d = nc.dram_tensor("wk", [KVDIM, 512], dt.bfloat16, kind="ExternalInput").ap()
    wv_d = nc.dram_tensor("wv", [KVDIM, 512], dt.bfloat16, kind="ExternalInput").ap()
    # indicator bits for the vsb denominator column: [128, hp, h, t]
    vind_d = nc.dram_tensor("vindb", [128, 4 * 2 * T], dt.bfloat16,
                            kind="ExternalInput").ap()
    # rows (hp*2+h)*65 .. +64: unnormalized AV.T ; row +64: denominator
    out_d = nc.dram_tensor("out", [520, ql], dt.float32, kind="ExternalOutput").ap()

    w_dram = {"wq": wq_d, "wk": wk_d, "wv": wv_d}
    bal = _Balance()

    with tile.TileContext(nc) as tc:
        with ExitStack() as ctx:
            persist = ctx.enter_context(tc.tile_pool(name="persist", bufs=1))

            qTin = [persist.tile([128, ql], dt.bfloat16, tag=f"qTin{j}", name=f"qTin{j}") for j in range(4)]
            kTin = [persist.tile([128, KC], dt.bfloat16, tag=f"kTin{j}", name=f"kTin{j}") for j in range(4)]
            vTin = [persist.tile([128, KC], dt.bfloat16, tag=f"vTin{j}", name=f"vTin{j}") for j in range(4)]
            wsb = {
                nm: [persist.tile([128, 512], dt.bfloat16, tag=f"{nm}{j}", name=f"{nm}{j}") for j in range(4)]
                for nm in ("wq", "wk", "wv")
            }
            qflat = [persist.tile([128, ql], dt.bfloat16, tag=f"qf{c}", name=f"qf{c}") for c in range(4)]
            kTz = [persist.tile([128, KC], dt.bfloat16, tag=f"kz{c}", name=f"kz{c}") for c in range(4)]
            # [v_h (64 dims) | indicator | pad to 72], per (h, t)
            vsb = [persist.tile([128, 2, T, 72], dt.bfloat16, tag=f"v{c}", name=f"v{c}") for c in range(4)]
            junk = persist.tile([128, 512], dt.bfloat16, tag="junk", name="junk")

            # ---- input DMAs (ordered so proj(0) can start early) ----
            for j in range(4):
                nc.sync.dma_start(wsb["wq"][j], w_dram["wq"][j * 128:(j + 1) * 128, :])
            for j in range(4):
                nc.sync.dma_start(qTin[j], qT_d[j * 128:(j + 1) * 128, :])
            for j in range(4):
                nc.sync.dma_start(wsb["wk"][j], w_dram["wk"][j * 128:(j + 1) * 128, :])
            for j in range(4):
                nc.sync.dma_start(kTin[j], kT_d[j * 128:(j + 1) * 128, :])
            for j in range(4):
                nc.sync.dma_start(wsb["wv"][j], w_dram["wv"][j * 128:(j + 1) * 128, :])
            for j in range(4):
                nc.sync.dma_start(vTin[j], vT_d[j * 128:(j + 1) * 128, :])
            for c in range(4):
                w = 2 * T
                nc.gpsimd.dma_start(
                    vsb[c][:, :, :, 64:65],
                    vind_d[:, c * w:(c + 1) * w].rearrange(
                        "p (h t) -> p h t", h=2).unsqueeze(3),
                )

            with tc.tile_pool(name="psE", bufs=3, space="PSUM") as psE, \
                 tc.tile_pool(name="psO", bufs=1, space="PSUM") as psO, \
                 tc.tile_pool(name="wxp", bufs=4) as wxp, \
                 tc.tile_pool(name="sbo", bufs=4) as sbo:

                # PE clock-gate warmup during the input-DMA window
                nc.vector.memset(junk, 1.0)
                for _ in range(4):
                    ps = psE.tile([128, 1024], dt.float32, tag="u", name="u")
                    for r in range(12):
                        nc.tensor.matmul(ps[:, 0:512], lhsT=junk[:, :128], rhs=junk,
                                         start=(r == 0), stop=(r == 11))

                def evac(out_ap, in_ap, fd):
                    if bal.pick(fd) == "act":
                        nc.scalar.copy(out_ap, in_ap)
                    else:
                        nc.vector.tensor_copy(out_ap, in_ap)

                def proj_chunks(c):
                    """Projection work for head-pair c as a list of closures."""
                    chunks = []

                    def qchunk(lo):
                        def emit():
                            ps = psE.tile([128, 1024], dt.float32, tag="u", name="u")
                            for half in range(2):
                                s = lo + half * 512
                                for j in range(4):
                                    nc.tensor.matmul(
                                        ps[:, half * 512:(half + 1) * 512],
                                        lhsT=wsb["wq"][j][:, c * 128:(c + 1) * 128],
                                        rhs=qTin[j][:, s:s + 512],
                                        start=(j == 0), stop=(j == 3),
                                    )
                            evac(qflat[c][:, lo:lo + 1024], ps[:, :], 1024)
                        return emit

                    def kchunk(lo, w):
                        def emit():
                            ps = psE.tile([128, 1024], dt.float32, tag="u", name="u")
                            for s in range(0, w, 512):
                                cw = min(512, w - s)
                                for j in range(4):
                                    nc.tensor.matmul(
                                        ps[:, s:s + cw],
                                        lhsT=wsb["wk"][j][:, c * 128:(c + 1) * 128],
                                        rhs=kTin[j][:, lo + s:lo + s + cw],
                                        start=(j == 0), stop=(j == 3),
                                    )
                            evac(kTz[c][:, lo:lo + w], ps[:, 0:w], w)
                        return emit

                    def vchunk(t0, nt):
                        def emit():
                            ps = psE.tile([128, 1024], dt.float32, tag="u", name="u")
                            for tt in range(nt):
                                for j in range(4):
                                    nc.tensor.matmul(
                                        ps[:, tt * 128:(tt + 1) * 128],
                                        lhsT=vTin[j][:, (t0 + tt) * 128:(t0 + tt + 1) * 128],
                                        rhs=wsb["wv"][j][:, c * 128:(c + 1) * 128],
                                        start=(j == 0), stop=(j == 3),
                                    )
                            # src [128, (tt, h, d)] -> dst vsb[h, t, d]
                            src = ps[:, 0:nt * 128].rearrange(
                                "p (t h d) -> p t h d", t=nt, h=2)
                            dst = vsb[c][:, :, t0:t0 + nt, 0:64].rearrange(
                                "p h t d -> p t h d")
                            evac(dst, src, nt * 128)
                        return emit

                    chunks.append(qchunk(0))
                    chunks.append(qchunk(1024))
                    ko = 0
                    while ko < KC:
                        w = min(1024, KC - ko)
                        chunks.append(kchunk(ko, w))
                        ko += w
                    to = 0
                    while to < T:
                        nt = min(8, T - to)
                        chunks.append(vchunk(to, nt))
                        to += nt
                    return chunks

                def attention_block(c, qb):
                    av = [psO.tile([65, 512], dt.float32, tag=f"av{h}", name=f"av{h}")
                          for h in range(2)]
                    q0 = qb * 512
                    for k in range(T):
                        u = psE.tile([128, 1024], dt.float32, tag="u", name="u")
                        nc.tensor.matmul(
                            u[:, 0:512],
                            lhsT=kTz[c][0:64, k * 128:(k + 1) * 128],
                            rhs=qflat[c][0:64, q0:q0 + 512],
                            start=True, stop=True)
                        nc.tensor.matmul(
                            u[:, 512:1024],
                            lhsT=kTz[c][64:128, k * 128:(k + 1) * 128],
                            rhs=qflat[c][64:128, q0:q0 + 512],
                            start=True, stop=True)
                        wx = wxp.tile([128, 2, 512], dt.bfloat16, tag="wx", name="wx")
                        src = u[:, :].rearrange("p (h q) -> p h q", h=2)
                        if bal.pick(1024) == "act":
                            nc.scalar.activation(
                                wx[:, :, :], src,
                                mybir.ActivationFunctionType.Exp, scale=SCALE)
                        else:
                            nc.vector.tensor_scalar(
                                wx[:, :, :].bitcast(dt.int16), src, S1_DVE, S2_DVE,
                                mybir.AluOpType.mult, mybir.AluOpType.add)
                        for h in range(2):
                            nc.tensor.matmul(
                                av[h],
                                lhsT=vsb[c][:, h, k, 0:65],
                                rhs=wx[:, h, :],
                                start=(k == 0), stop=(k == T - 1))
                    for h in range(2):
                        ot = sbo.tile([65, 512], dt.float32, tag="ot", name="ot")
                        evac(ot, av[h][:, :], 512)
                        nc.gpsimd.dma_start(
                            out_d[(c * 2 + h) * 65:(c * 2 + h) * 65 + 65,
                                  q0:q0 + 512], ot)

                # ---- main schedule: proj(0), then per head-pair c:
                # attention blocks interleaved with proj(c+1) chunks ----
                for ch in proj_chunks(0):
                    ch()
                for c in range(4):
                    nxt = proj_chunks(c + 1) if c < 3 else []
                    per = (len(nxt) + NQB - 1) // NQB if nxt else 0
                    for qb in range(NQB):
                        for ch in nxt[qb * per:(qb + 1) * per]:
                            ch()
                        attention_block(c, qb)

    nc.compile()
    return nc


def _prepare(queries, keys, values, mask):
    """Host-side sharding: transpose, compact kv by mask, indicator tiles."""
    m = np.asarray(mask).reshape(N, KLEN) != 0
    idx = [np.nonzero(m[n])[0] for n in range(N)]
    cnts = [len(i) for i in idx]
    T = max(2, (max(cnts) + 127) // 128)
    KC = 128 * T

    kT_full = np.ascontiguousarray(np.asarray(keys, np.float32)[0].T)
    vT_full = np.ascontiguousarray(np.asarray(values, np.float32)[0].T)
    q32 = np.asarray(queries, np.float32)

    qT_n, kT_n, vT_n, vind_n = [], [], [], []
    for n in range(N):
        kt = np.zeros((KVDIM, KC), np.float32)
        vt = np.zeros((KVDIM, KC), np.float32)
        kt[:, :cnts[n]] = kT_full[:, idx[n]]
        vt[:, :cnts[n]] = vT_full[:, idx[n]]
        # indicator per (partition p, hp, h, t): token t*128+p valid?
        ind = (np.arange(KC) < cnts[n]).astype(np.float32).reshape(T, 128).T
        v8 = np.broadcast_to(ind[:, None, None, :], (128, 4, 2, T))
        vind_n.append(np.ascontiguousarray(v8.reshape(128, -1)).astype(BF16))
        kT_n.append(kt.astype(BF16))
        vT_n.append(vt.astype(BF16))
        qT_n.append(np.ascontiguousarray(q32[n].T).astype(BF16))
    return T, qT_n, kT_n, vT_n, vind_n


def kernel(queries, keys, values, mask, Wq, Wk, Wv, _trace=False):
    global last_exec_time_ns, last_results
    T, qT_n, kT_n, vT_n, vind_n = _prepare(queries, keys, values, mask)

    w_g = {}
    for nm, W in (("wq", Wq), ("wk", Wk), ("wv", Wv)):
        W = np.asarray(W, np.float32)
        w_g[nm] = [np.ascontiguousarray(W[:, g * 512:(g + 1) * 512]).astype(BF16)
                   for g in range(2)]

    nc = _cache.get(T)
    if nc is None:
        nc = _cache.setdefault(T, _build(T))

    in_maps = []
    for core in range(N_CORES):
        n, g = core // 2, core % 2
        in_maps.append({
            "qt": qT_n[n], "kt": kT_n[n], "vt": vT_n[n],
            "wq": w_g["wq"][g], "wk": w_g["wk"][g], "wv": w_g["wv"][g],
            "vindb": vind_n[n],
        })

    res = run_bass_kernel_spmd(nc, in_maps, core_ids=list(range(N_CORES)),
                               trace=bool(_trace))
    last_exec_time_ns = res.exec_time_ns
    last_results = res

    full = np.empty((N, QLEN, EMBED), np.float32)
    for core in range(N_CORES):
        n, g = core // 2, core % 2
        o = res.results[core]["out"].reshape(8, 65, QLEN)
        vals = o[:, :64, :] / o[:, 64:65, :]          # [8, 64, QLEN]
        full[n, :, g * 512:(g + 1) * 512] = (
            vals.transpose(2, 0, 1).reshape(QLEN, 512)
        )
    return full


# revision 8
# speedup vs baseline: 1.0508x; 1.0508x over previous
"""MultiHeadCrossAttention Trainium2 kernel (8 NeuronCores, SPMD).

Sharding: core c -> (n = c // 2, g = c % 2): one query batch n, half the
heads (8 of 16, embed slice g*512:(g+1)*512). Host compacts kv along KLEN
by the per-n mask (~50% survive), pads to KC = 128*T.

Key insight driving dtype choices: attention output is a softmax-weighted
AVERAGE of v, whose magnitude shrinks by the same sqrt(Neff) as the
weight-noise averaging gain -- so final rel err ~= per-weight RMS error.
fp8 weights (3-5% RMS) can never pass rel<2e-2; everything on the weight
and value paths stays bf16 (~0.2-1.8% RMS).

Per-core design (v3):
  - Projections (bf16, K=512 via 4 accumulating matmuls) flow through the
    shared PSUM pipeline; outputs evacuated fp32->bf16 by whichever of
    ScalarE/VectorE is currently less loaded (greedy balance).
  - Energy e[k,q] = kT.T @ qT per head at K=64 with NO zero padding: the
    two heads of a pair run as row-tiled concurrent matmuls (rows 0-63 /
    64-127 of the PE array) -> 2x the padded-K=128 baseline throughput.
  - exp is split across BOTH ScalarE and VectorE (the bottleneck is
    PSUM->SBUF evacuation bandwidth, exp included):
      * ScalarE: native Exp ACTIVATE, fp32 PSUM -> bf16 SBUF.
      * VectorE: one-op Schraudolph fast-exp: i16 = round(e*S1 + S2)
        written as int16 bits that ARE the bf16 encoding of ~exp(e/8)
        (fp32->int convert is round-to-nearest on HW; ~1.8% RMS, mean
        calibrated to zero via S2). One tensor_scalar per unit.
  - AV: per k-tile bf16 matmuls, lhsT = vsb [Ki=128 tokens, M=65] =
    [64 v-dims | indicator], indicator row accumulates the softmax
    denominator; pad tokens have v=0, ind=0. M=65 costs nothing (matmul
    time is set by N).
  - Unnormalized AV + denominator row DMA'd out; host divides/assembles.
"""

import math
import sys
from contextlib import ExitStack

import numpy as np

for _p in ("/opt/trn_rl_repo",):
    if _p not in sys.path:
        sys.path.insert(0, _p)

import ml_dtypes

import concourse.bass as bass  # noqa: F401  (import registers lowering deps)
import concourse.tile as tile
from concourse import bacc, mybir
from concourse.bass_utils import run_bass_kernel_spmd

BF16 = ml_dtypes.bfloat16

N, QLEN, KLEN = 4, 2048, 2048
QDIM = KVDIM = 512
EMBED, HEADS = 1024, 16
HEAD_DIM = 64
N_CORES = 8
SCALE = 1.0 / math.sqrt(HEAD_DIM)  # 1/8
# VectorE Schraudolph constants for bf16 bits (128/octave, bias 127):
#   i16 = round(e * (128/ln2)/8 + (127*128 - c))), c calibrated so the
#   piecewise-linear overestimate is mean-zero.
S1_DVE = (128.0 / math.log(2.0)) / 8.0
S2_DVE = 127.0 * 128.0 - 7.37

_cache: dict = {}
last_exec_time_ns = None
last_results = None


class _Balance:
    """Greedy ScalarE/VectorE assignment by simulated busy time."""

    def __init__(self):
        self.t_act = 0.0
        self.t_dve = 0.0

    @staticmethod
    def cost_act(fd):
        return (172.0 + fd) / 1.2

    @staticmethod
    def cost_dve(fd):
        return (120.0 + fd) / 0.96

    def pick(self, fd):
        ca = self.cost_act(fd)
        cd = self.cost_dve(fd)
        if self.t_act + ca <= self.t_dve + cd:
            self.t_act += ca
            return "act"
        self.t_dve += cd
        return "dve"


def _build(T: int, ql: int = QLEN):
    KC = 128 * T
    NQB = ql // 512
    dt = mybir.dt
    nc = bacc.Bacc("TRN2", target_bir_lowering=False, debug=False)

    qT_d = nc.dram_tensor("qt", [QDIM, ql], dt.bfloat16, kind="ExternalInput").ap()
    kT_d = nc.dram_tensor("kt", [KVDIM, KC], dt.bfloat16, kind="ExternalInput").ap()
    vT_d = nc.dram_tensor("vt", [KVDIM, KC], dt.bfloat16, kind="ExternalInput").ap()
    wq_d = nc.dram_tensor("wq", [QDIM, 512], dt.bfloat16, kind="ExternalInput").ap()
    wk_d = nc.dram_tensor("wk", [KVDIM, 512], dt.bfloat16, kind="ExternalInput").ap()
    wv_d = nc.dram_tensor("wv", [KVDIM, 512], dt.bfloat16, kind="ExternalInput").ap()
    # indicator bits for the vsb denominator column: [128, hp, h, t]
    vind_d = nc.dram_tensor("vindb", [128, 4 * 2 * T], dt.bfloat16,
                            kind="ExternalInput").ap()
    # rows (hp*2+h)*65 .. +64: unnormalized AV.T ; row +64: denominator
    out_d = nc.dram_tensor("out", [520, ql], dt.float32, kind="ExternalOutput").ap()

    w_dram = {"wq": wq_d, "wk": wk_d, "wv": wv_d}
    bal = _Balance()

    with tile.TileContext(nc) as tc:
        with ExitStack() as ctx:
            persist = ctx.enter_context(tc.tile_pool(name="persist", bufs=1))

            qTin = [persist.tile([128, ql], dt.bfloat16, tag=f"qTin{j}", name=f"qTin{j}") for j in range(4)]
            kTin = [persist.tile([128, KC], dt.bfloat16, tag=f"kTin{j}", name=f"kTin{j}") for j in range(4)]
            vTin = [persist.tile([128, KC], dt.bfloat16, tag=f"vTin{j}", name=f"vTin{j}") for j in range(4)]
            wsb = {
                nm: [persist.tile([128, 512], dt.bfloat16, tag=f"{nm}{j}", name=f"{nm}{j}") for j in range(4)]
                for nm in ("wq", "wk", "wv")
            }
            qflat = [persist.tile([128, ql], dt.bfloat16, tag=f"qf{c}", name=f"qf{c}") for c in range(4)]
            kTz = [persist.tile([128, KC], dt.bfloat16, tag=f"kz{c}", name=f"kz{c}") for c in range(4)]
            # [v_h (64 dims) | indicator | pad to 72], per (h, t)
            vsb = [persist.tile([128, 2, T, 72], dt.bfloat16, tag=f"v{c}", name=f"v{c}") for c in range(4)]
            junk = persist.tile([128, 512], dt.bfloat16, tag="junk", name="junk")

            # ---- input DMAs (ordered so proj(0) can start early) ----
            for j in range(4):
                nc.sync.dma_start(wsb["wq"][j], w_dram["wq"][j * 128:(j + 1) * 128, :])
            for j in range(4):
                nc.sync.dma_start(qTin[j], qT_d[j * 128:(j + 1) * 128, :])
            for j in range(4):
                nc.sync.dma_start(wsb["wk"][j], w_dram["wk"][j * 128:(j + 1) * 128, :])
            for j in range(4):
                nc.sync.dma_start(kTin[j], kT_d[j * 128:(j + 1) * 128, :])
            for j in range(4):
                nc.sync.dma_start(wsb["wv"][j], w_dram["wv"][j * 128:(j + 1) * 128, :])
            for j in range(4):
                nc.sync.dma_start(vTin[j], vT_d[j * 128:(j + 1) * 128, :])
            for c in range(4):
                w = 2 * T
                nc.gpsimd.dma_start(
                    vsb[c][:, :, :, 64:65],
                    vind_d[:, c * w:(c + 1) * w].rearrange(
                        "p (h t) -> p h t", h=2).unsqueeze(3),
                )

            with tc.tile_pool(name="psE", bufs=3, space="PSUM") as psE, \
                 tc.tile_pool(name="psO", bufs=1, space="PSUM") as psO, \
                 tc.tile_pool(name="wxp", bufs=4) as wxp, \
                 tc.tile_pool(name="sbo", bufs=4) as sbo:

                # PE clock-gate warmup during the input-DMA window
                nc.vector.memset(junk, 1.0)
                for _ in range(4):
                    ps = psE.tile([128, 1024], dt.float32, tag="u", name="u")
                    for r in range(12):
                        nc.tensor.matmul(ps[:, 0:512], lhsT=junk[:, :128], rhs=junk,
                                         start=(r == 0), stop=(r == 11))

                def evac(out_ap, in_ap, fd):
                    if bal.pick(fd) == "act":
                        nc.scalar.copy(out_ap, in_ap)
                    else:
                        nc.vector.tensor_copy(out_ap, in_ap)

                def proj_chunks(c):
                    """Projection work for head-pair c as a list of closures."""
                    chunks = []

                    def qchunk(lo):
                        def emit():
                            ps = psE.tile([128, 1024], dt.float32, tag="u", name="u")
                            for half in range(2):
                                s = lo + half * 512
                                for j in range(4):
                                    nc.tensor.matmul(
                                        ps[:, half * 512:(half + 1) * 512],
                                        lhsT=wsb["wq"][j][:, c * 128:(c + 1) * 128],
                                        rhs=qTin[j][:, s:s + 512],
                                        start=(j == 0), stop=(j == 3),
                                    )
                            evac(qflat[c][:, lo:lo + 1024], ps[:, :], 1024)
                        return emit

                    def kchunk(lo, w):
                        def emit():
                            ps = psE.tile([128, 1024], dt.float32, tag="u", name="u")
                            for s in range(0, w, 512):
                                cw = min(512, w - s)
                                for j in range(4):
                                    nc.tensor.matmul(
                                        ps[:, s:s + cw],
                                        lhsT=wsb["wk"][j][:, c * 128:(c + 1) * 128],
                                        rhs=kTin[j][:, lo + s:lo + s + cw],
                                        start=(j == 0), stop=(j == 3),
                                    )
                            evac(kTz[c][:, lo:lo + w], ps[:, 0:w], w)
                        return emit

                    def vchunk(t0, nt):
                        def emit():
                            ps = psE.tile([128, 1024], dt.float32, tag="u", name="u")
                            for tt in range(nt):
                                for j in range(4):
                                    nc.tensor.matmul(
                                        ps[:, tt * 128:(tt + 1) * 128],
                                        lhsT=vTin[j][:, (t0 + tt) * 128:(t0 + tt + 1) * 128],
                                        rhs=wsb["wv"][j][:, c * 128:(c + 1) * 128],
                                        start=(j == 0), stop=(j == 3),
                                    )
                            # src [128, (tt, h, d)] -> dst vsb[h, t, d]
                            src = ps[:, 0:nt * 128].rearrange(
                                "p (t h d) -> p t h d", t=nt, h=2)
                            dst = vsb[c][:, :, t0:t0 + nt, 0:64].rearrange(
                                "p h t d -> p t h d")
                            evac(dst, src, nt * 128)
                        return emit

                    chunks.append(qchunk(0))
                    chunks.append(qchunk(1024))
                    ko = 0
                    while ko < KC:
                        w = min(1024, KC - ko)
                        chunks.append(kchunk(ko, w))
                        ko += w
                    to = 0
                    while to < T:
                        nt = min(8, T - to)
                        chunks.append(vchunk(to, nt))
                        to += nt
                    return chunks

                # software pipeline: AV of unit i is emitted after E of unit
                # i+2, so the PE never waits on the exp engines (baseline's
                # prev-item trick, across qb/head-pair boundaries too)
                pend = []

                def flush_one():
                    c, qb, k, av, wx = pend.pop(0)
                    q0 = qb * 512
                    for h in range(2):
                        nc.tensor.matmul(
                            av[h],
                            lhsT=vsb[c][:, h, k, 0:65],
                            rhs=wx[:, h, :],
                            start=(k == 0), stop=(k == T - 1))
                    if k == T - 1:
                        for h in range(2):
                            ot = sbo.tile([65, 512], dt.float32, tag="ot", name="ot")
                            evac(ot, av[h][:, :], 512)
                            nc.gpsimd.dma_start(
                                out_d[(c * 2 + h) * 65:(c * 2 + h) * 65 + 65,
                                      q0:q0 + 512], ot)

                def attention_block(c, qb):
                    av = [psO.tile([65, 512], dt.float32, tag=f"av{h}", name=f"av{h}")
                          for h in range(2)]
                    q0 = qb * 512
                    for k in range(T):
                        u = psE.tile([128, 1024], dt.float32, tag="u", name="u")
                        nc.tensor.matmul(
                            u[:, 0:512],
                            lhsT=kTz[c][0:64, k * 128:(k + 1) * 128],
                            rhs=qflat[c][0:64, q0:q0 + 512],
                            start=True, stop=True)
                        nc.tensor.matmul(
                            u[:, 512:1024],
                            lhsT=kTz[c][64:128, k * 128:(k + 1) * 128],
                            rhs=qflat[c][64:128, q0:q0 + 512],
                            start=True, stop=True)
                        wx = wxp.tile([128, 2, 512], dt.bfloat16, tag="wx", name="wx")
                        src = u[:, :].rearrange("p (h q) -> p h q", h=2)
                        if bal.pick(1024) == "act":
                            nc.scalar.activation(
                                wx[:, :, :], src,
                                mybir.ActivationFunctionType.Exp, scale=SCALE)
                        else:
                            nc.vector.tensor_scalar(
                                wx[:, :, :].bitcast(dt.int16), src, S1_DVE, S2_DVE,
                                mybir.AluOpType.mult, mybir.AluOpType.add)
                        pend.append((c, qb, k, av, wx))
                        while len(pend) > 2:
                            flush_one()

                # ---- main schedule: proj(0), then per head-pair c:
                # attention blocks interleaved with proj(c+1) chunks ----
                for ch in proj_chunks(0):
                    ch()
                for c in range(4):
                    nxt = proj_chunks(c + 1) if c < 3 else []
                    per = (len(nxt) + NQB - 1) // NQB if nxt else 0
                    for qb in range(NQB):
                        for ch in nxt[qb * per:(qb + 1) * per]:
                            ch()
                        attention_block(c, qb)
                while pend:
                    flush_one()

    nc.compile()
    return nc


def _prepare(queries, keys, values, mask):
    """Host-side sharding: transpose, compact kv by mask, indicator tiles."""
    m = np.asarray(mask).reshape(N, KLEN) != 0
    idx = [np.nonzero(m[n])[0] for n in range(N)]
    cnts = [len(i) for i in idx]
    T = max(2, (max(cnts) + 127) // 128)
    KC = 128 * T

    kT_full = np.ascontiguousarray(np.asarray(keys, np.float32)[0].T)
    vT_full = np.ascontiguousarray(np.asarray(values, np.float32)[0].T)
    q32 = np.asarray(queries, np.float32)

    qT_n, kT_n, vT_n, vind_n = [], [], [], []
    for n in range(N):
        kt = np.zeros((KVDIM, KC), np.float32)
        vt = np.zeros((KVDIM, KC), np.float32)
        kt[:, :cnts[n]] = kT_full[:, idx[n]]
        vt[:, :cnts[n]] = vT_full[:, idx[n]]
        # indicator per (partition p, hp, h, t): token t*128+p valid?
        ind = (np.arange(KC) < cnts[n]).astype(np.float32).reshape(T, 128).T
        v8 = np.broadcast_to(ind[:, None, None, :], (128, 4, 2, T))
        vind_n.append(np.ascontiguousarray(v8.reshape(128, -1)).astype(BF16))
        kT_n.append(kt.astype(BF16))
        vT_n.append(vt.astype(BF16))
        qT_n.append(np.ascontiguousarray(q32[n].T).astype(BF16))
    return T, qT_n, kT_n, vT_n, vind_n


def kernel(queries, keys, values, mask, Wq, Wk, Wv, _trace=False):
    global last_exec_time_ns, last_results
    T, qT_n, kT_n, vT_n, vind_n = _prepare(queries, keys, values, mask)

    w_g = {}
    for nm, W in (("wq", Wq), ("wk", Wk), ("wv", Wv)):
        W = np.asarray(W, np.float32)
        w_g[nm] = [np.ascontiguousarray(W[:, g * 512:(g + 1) * 512]).astype(BF16)
                   for g in range(2)]

    nc = _cache.get(T)
    if nc is None:
        nc = _cache.setdefault(T, _build(T))

    in_maps = []
    for core in range(N_CORES):
        n, g = core // 2, core % 2
        in_maps.append({
            "qt": qT_n[n], "kt": kT_n[n], "vt": vT_n[n],
            "wq": w_g["wq"][g], "wk": w_g["wk"][g], "wv": w_g["wv"][g],
            "vindb": vind_n[n],
        })

    res = run_bass_kernel_spmd(nc, in_maps, core_ids=list(range(N_CORES)),
                               trace=bool(_trace))
    last_exec_time_ns = res.exec_time_ns
    last_results = res

    full = np.empty((N, QLEN, EMBED), np.float32)
    for core in range(N_CORES):
        n, g = core // 2, core % 2
        o = res.results[core]["out"].reshape(8, 65, QLEN)
        vals = o[:, :64, :] / o[:, 64:65, :]          # [8, 64, QLEN]
        full[n, :, g * 512:(g + 1) * 512] = (
            vals.transpose(2, 0, 1).reshape(QLEN, 512)
        )
    return full
